# revision 1
# baseline (speedup 1.0000x reference)
"""Bilinear pair-interaction layer on 8 Trainium2 NeuronCores.

reference: proj[b,p,:] = v_i[b] @ W[p]^T ; out = proj * v_j
  feature_emb [B=2048, F=32, D=128] f32, W [P=496, 128, 128] f32
  out [B, P, D] f32.

Sharding: 4 batch blocks x 2 pair groups (8 cores). Core m handles batch
rows [(m%4)*512, +512) and fields i of parity g = m//4 (pairs (i, j), j>i).
All cores run ONE identical instruction stream over a padded schedule
(slot ii <-> field i = 2*ii+g, njs(ii) = 31-2*ii local pairs); the
group-dependence lives entirely in per-core packed DRAM contents:
  emb_t: v_i^T for the core's own fields packed by slot ii      [d, 16, bs]
  emb_n: natural-layout emb with fields shifted by g            [bs, F, d]
         (slot f holds field f+g, so the static multiplier slice
          2*ii+1+k addresses field i+1+k of THIS core's group)
  w_t:   transposed pair weights packed by local pair slot       [d, Q, d]
Odd-group pads (one slot per ii, W=0) produce zero columns the host drops.

Per slot ii: one DMA brings njs pair weights; per batch chunk c (4x128):
stationary lhsT = v_i^T chunk, one matmul per 4 pairs (N=512) into
PSUM, DVE multiplies by v_j, one DMA per (ii,c) writes the contiguous
local pair range. Steady-state DMA is ~84MB/core/run vs 65MB output:
close to the ~320GB/s per-core HBM ceiling.
"""

import numpy as np

import concourse.bass as bass
import concourse.tile as tile
from concourse import bacc, mybir
from concourse.bass_utils import run_bass_kernel_spmd

B, F, D = 2048, 32, 128
P = F * (F - 1) // 2  # 496
N_CORES = 8
NBB = 4               # batch blocks
NGRP = 2              # pair (field-parity) groups
BS = B // NBB         # 512 batch rows per core
NCHUNK = BS // 128    # 4 partition chunks
GRP = 4               # pairs per matmul (N = GRP*128 = 512)
NSLOT = F // 2        # 16 schedule slots (field i = 2*ii + g)


def _njs(ii: int) -> int:
    # padded per-slot pair count (group-0 sizes; group 1 pads)
    return F - 1 - 2 * ii


Q = sum(_njs(ii) for ii in range(NSLOT))  # 256 local pairs per core


def _q0(ii: int) -> int:
    return sum(_njs(k) for k in range(ii))


def _pair_offset(i: int) -> int:
    # first global pair index for field i (itertools.combinations order)
    return i * (2 * F - i - 1) // 2


def build(nc, repeat: int = 1, timing: bool = False, mm_dtype: str = "float32r",
          mode: str = "full"):
    f32 = mybir.dt.float32
    mmdt = getattr(mybir.dt, mm_dtype)
    if timing:
        # timing-only build: no big external I/O (host<->device shipping would
        # swamp wall-clock); kernel reads/writes internal DRAM scratch.
        emb_t = nc.dram_tensor("emb_t", [D, NSLOT, BS], mmdt)
        emb_n = nc.dram_tensor("emb_n", [BS, F, D], f32)
        w_t = nc.dram_tensor("w_t", [D, Q, D], mmdt)
        out = nc.dram_tensor("out", [BS, Q, D], f32)
        tok_in = nc.dram_tensor("tok_in", [1, 4], f32, kind="ExternalInput")
        tok_out = nc.dram_tensor("tok_out", [1, 4], f32, kind="ExternalOutput")
    else:
        emb_t = nc.dram_tensor("emb_t", [D, NSLOT, BS], mmdt, kind="ExternalInput")
        emb_n = nc.dram_tensor("emb_n", [BS, F, D], f32, kind="ExternalInput")
        w_t = nc.dram_tensor("w_t", [D, Q, D], mmdt, kind="ExternalInput")
        out = nc.dram_tensor("out", [BS, Q, D], f32, kind="ExternalOutput")
        tok_in = tok_out = None

    with tile.TileContext(nc) as tc:
        with (
            tc.tile_pool(name="embt", bufs=1) as embt_pool,
            tc.tile_pool(name="embn", bufs=1) as embn_pool,
            tc.tile_pool(name="wt", bufs=3) as w_pool,
            tc.tile_pool(name="stage", bufs=3) as stage_pool,
            tc.tile_pool(name="psum", bufs=8, space="PSUM") as psum_pool,
        ):
            if timing:
                # zero-fill internal scratch inputs so matmuls see no
                # NaNs/denormals (garbage DRAM could perturb timing)
                with tc.tile_pool(name="zt", bufs=1) as z_pool:
                    zt = z_pool.tile([128, 4096], f32)
                    nc.vector.memset(zt[:], 0.0)
                    wt_flat = w_t.ap().rearrange("d p e -> d (p e)").bitcast(f32)
                    for k in range(Q * D // 4096):
                        nc.sync.dma_start(wt_flat[:, k * 4096 : (k + 1) * 4096], zt[:])
                    et_flat = emb_t.ap().rearrange("d s b -> d (s b)").bitcast(f32)
                    for k in range(NSLOT * BS // 4096):
                        nc.sync.dma_start(et_flat[:, k * 4096 : (k + 1) * 4096], zt[:])
                    en_flat = emb_n.ap().rearrange("(c b) f d -> b c (f d)", b=128)
                    for c in range(NCHUNK):
                        nc.sync.dma_start(en_flat[:, c, :], zt[:])

            # whole-core emb resident in SBUF (loaded once, outside repeat loop)
            embt = embt_pool.tile([128, NSLOT * BS], mmdt)
            nc.sync.dma_start(embt[:], emb_t.ap().rearrange("d s b -> d (s b)"))
            embn = embn_pool.tile([128, NCHUNK * F * D], f32)
            for c in range(NCHUNK):
                nc.sync.dma_start(
                    embn[:, c * F * D : (c + 1) * F * D],
                    emb_n.ap()[c * 128 : (c + 1) * 128, :, :]
                    .rearrange("b f d -> b (f d)"),
                )

            def body(_iv=None):
                for ii in range(NSLOT):
                    njs = _njs(ii)
                    q0 = _q0(ii)
                    wt = w_pool.tile([128, _njs(0) * D], mmdt, tag="wt")
                    nc.sync.dma_start(
                        wt[:, : njs * D],
                        w_t.ap()[:, q0 : q0 + njs, :].rearrange("d p e -> d (p e)"),
                    )
                    for c in range(NCHUNK):
                        lhsT = embt[:, ii * BS + c * 128 : ii * BS + c * 128 + 128]
                        if mode != "dma":
                            stage = stage_pool.tile(
                                [128, _njs(0) * D], f32, tag="st"
                            )
                        for k0 in range(0, njs, GRP):
                            if mode == "dma":
                                continue
                            g = min(GRP, njs - k0)
                            ps = psum_pool.tile([128, GRP * D], f32)
                            nc.tensor.matmul(
                                ps[:, : g * D],
                                lhsT,
                                wt[:, k0 * D : (k0 + g) * D],
                                start=True,
                                stop=True,
                            )
                            if mode == "pe":
                                continue
                            j0 = 2 * ii + 1 + k0  # field slot (core-local)
                            emb_j = embn[
                                :, c * F * D + j0 * D : c * F * D + (j0 + g) * D
                            ]
                            nc.vector.tensor_mul(
                                stage[:, k0 * D : (k0 + g) * D],
                                ps[:, : g * D],
                                emb_j,
                            )
                        if mode == "full":
                            src_ap = stage[:, : njs * D]
                        elif mode == "dma":
                            src_ap = embn[:, : njs * D]  # any resident SBUF data
                        else:
                            src_ap = None
                        if src_ap is not None:
                            dma_eng = nc.scalar if (ii % 2) else nc.sync
                            dma_eng.dma_start(
                                out.ap()[c * 128 : (c + 1) * 128, q0 : q0 + njs, :]
                                .rearrange("b p e -> b (p e)"),
                                src_ap,
                            )

            if repeat == 1:
                body()
            else:
                with tc.For_i(
                    0,
                    repeat,
                    1,
                    hint_engines=(
                        mybir.EngineType.PE,
                        mybir.EngineType.DVE,
                        mybir.EngineType.SP,
                    ),
                ) as _i:
                    body(_i)

            if timing:
                st = stage_pool.tile([128, _njs(0) * D], f32, tag="st")
                nc.sync.dma_start(st[:1, :4], tok_in.ap())
                nc.sync.dma_start(tok_out.ap(), st[:1, :4])
    return nc


_NC_CACHE = {}
MM_DTYPE = "float32r"  # fp32r: 4x PE throughput; measured resid_var ~1.7e-8


def _get_nc(repeat: int = 1, timing: bool = False, mm_dtype: str | None = None,
            mode: str = "full"):
    mm_dtype = mm_dtype or MM_DTYPE
    key = (repeat, timing, mm_dtype, mode)
    if key not in _NC_CACHE:
        nc = bacc.Bacc("TRN2", target_bir_lowering=False, debug=False)
        build(nc, repeat=repeat, timing=timing, mm_dtype=mm_dtype, mode=mode)
        nc.compile()
        _NC_CACHE[key] = nc
    return _NC_CACHE[key]


def make_in_maps(feature_emb: np.ndarray, W: np.ndarray):
    feature_emb = np.ascontiguousarray(np.asarray(feature_emb, dtype=np.float32))
    W = np.asarray(W, dtype=np.float32)
    w_all = np.ascontiguousarray(W.transpose(2, 0, 1))  # [d, p_global, e]

    # per-group local-pair -> global-pair map (-1 = pad slot)
    pair_map = {}
    for g in range(NGRP):
        pm = np.full(Q, -1, dtype=np.int64)
        for ii in range(NSLOT):
            i = 2 * ii + g
            if i > F - 2:
                continue
            nja = min(F - 1 - i, _njs(ii))  # actual pairs for this field
            q0 = _q0(ii)
            gp0 = _pair_offset(i)
            pm[q0 : q0 + nja] = gp0 + np.arange(nja)
        pair_map[g] = pm

    w_loc = {}
    for g in range(NGRP):
        wl = np.zeros((D, Q, D), dtype=np.float32)
        valid = pair_map[g] >= 0
        wl[:, valid, :] = w_all[:, pair_map[g][valid], :]
        w_loc[g] = wl

    in_maps = []
    for m in range(N_CORES):
        bb, g = m % NBB, m // NBB
        emb_m = feature_emb[bb * BS : (bb + 1) * BS]  # [bs, f, d]
        # emb_n: fields shifted by g so static slot f holds field f+g
        en = np.empty_like(emb_m)
        en[:, : F - g, :] = emb_m[:, g:, :]
        if g:
            en[:, F - g :, :] = emb_m[:, -1:, :]  # pad slot (result discarded)
        # emb_t: core's own fields (i = 2*ii+g) packed by slot, transposed
        et = np.zeros((D, NSLOT, BS), dtype=np.float32)
        for ii in range(NSLOT):
            i = 2 * ii + g
            if i <= F - 2:
                et[:, ii, :] = emb_m[:, i, :].T
        in_maps.append(
            {
                "emb_t": et,
                "emb_n": np.ascontiguousarray(en),
                "w_t": w_loc[g],
            }
        )
    return in_maps


def gather_out(results):
    out = np.empty((B, P, D), dtype=np.float32)
    for m in range(N_CORES):
        bb, g = m % NBB, m // NBB
        loc = results[m]["out"]  # [bs, Q, d]
        rows = slice(bb * BS, (bb + 1) * BS)
        for ii in range(NSLOT):
            i = 2 * ii + g
            if i > F - 2:
                continue
            nja = min(F - 1 - i, _njs(ii))
            q0 = _q0(ii)
            gp0 = _pair_offset(i)
            out[rows, gp0 : gp0 + nja, :] = loc[:, q0 : q0 + nja, :]
    return out


def run(in_maps, repeat: int = 1, timing: bool = False, mm_dtype: str | None = None,
        mode: str = "full"):
    nc = _get_nc(repeat, timing, mm_dtype, mode)
    return run_bass_kernel_spmd(nc, in_maps, list(range(N_CORES)))


def run_timing(repeat: int, mm_dtype: str | None = None, mode: str = "full"):
    tok = np.zeros((1, 4), np.float32)
    return run([{"tok_in": tok} for _ in range(N_CORES)], repeat=repeat,
               timing=True, mm_dtype=mm_dtype, mode=mode)


def kernel(feature_emb: np.ndarray, W: np.ndarray) -> np.ndarray:
    res = run(make_in_maps(feature_emb, W))
    return gather_out(res.results)

